# revision 1
# baseline (speedup 1.0000x reference)
"""CrossMatchingLoss Trainium2 kernel.

Problem: loss = -mean(matched cosine sims) where the matching is an exact
Hungarian assignment per batch element (detached / solved on CPU, exactly as
the reference does).

Split of work:
  host : L2-normalize (f32), transpose features to (D, N) layout, cast bf16,
         shard batches 4-per-core across 8 NeuronCores.
  device (per core): sim[b] = s_hat[b] @ t_hat[b]^T for its 4 batches as
         bf16 matmuls with f32 PSUM accumulation -> bf16 sim matrices.
  host : Hungarian assignment per batch on the device sim, then gather the
         matched cosine values (recomputed in f32/f64 for exactness) and
         average.

Shapes are hardcoded for B=32, N=256, D=1024, 8 cores (4 batches per core).
"""

import numpy as np
import ml_dtypes

B, N, D = 32, 256, 1024
N_CORES = 8
BPC = B // N_CORES          # batches per core
KC = D // 128               # contraction k-chunks of 128

_NC_CACHE = {}


def _build_bass():
    """Build (once) the per-core Bass program: 4x [256,1024]@[1024,256]^T."""
    if "nc" in _NC_CACHE:
        return _NC_CACHE["nc"]

    import concourse.bacc as bacc
    import concourse.mybir as mybir
    import concourse.tile as tile

    nc = bacc.Bacc("TRN2")
    x = nc.dram_tensor(
        "x", [BPC, 2, D, N], mybir.dt.bfloat16, kind="ExternalInput"
    )
    sim = nc.dram_tensor(
        "sim", [BPC, N, N], mybir.dt.bfloat16, kind="ExternalOutput"
    )

    with tile.TileContext(nc) as tc:
        with (
            tc.tile_pool(name="xin", bufs=2) as xin_pool,
            tc.tile_pool(name="so", bufs=2) as so_pool,
            tc.tile_pool(name="ps", bufs=4, space="PSUM") as ps_pool,
        ):
            for b in range(BPC):
                xt = xin_pool.tile([128, 2, KC, N], mybir.dt.bfloat16)
                # (st, d, n) -> partition = d % 128, free = (st, d//128, n)
                nc.sync.dma_start(
                    out=xt[:],
                    in_=x[b].rearrange("st (k p) n -> p st k n", k=KC, p=128),
                )
                so = so_pool.tile([128, 2, N], mybir.dt.bfloat16)
                for j in range(2):
                    ps = ps_pool.tile([128, N], mybir.dt.float32)
                    for k in range(KC):
                        nc.tensor.matmul(
                            ps[:],
                            xt[:, 0, k, 128 * j : 128 * (j + 1)],  # lhsT [d,n]
                            xt[:, 1, k, :],                        # rhs  [d,m]
                            start=(k == 0),
                            stop=(k == KC - 1),
                        )
                    nc.vector.tensor_copy(so[:, j, :], ps[:])
                nc.scalar.dma_start(
                    out=sim[b].rearrange("(j p) m -> p j m", j=2, p=128),
                    in_=so[:],
                )

    nc.compile()
    _NC_CACHE["nc"] = nc
    return nc


def _l2norm(x):
    n = np.sqrt(np.sum(np.square(x), axis=-1, keepdims=True, dtype=np.float32))
    return x / np.maximum(n, np.float32(1e-12))


def _hungarian_min(cost):
    """Exact square linear_sum_assignment (minimize); numpy fallback."""
    n = cost.shape[0]
    c = np.asarray(cost, dtype=np.float64)
    INF = np.inf
    u = np.zeros(n + 1)
    v = np.zeros(n + 1)
    p = np.zeros(n + 1, dtype=np.int64)
    way = np.zeros(n + 1, dtype=np.int64)
    for i in range(1, n + 1):
        p[0] = i
        j0 = 0
        minv = np.full(n + 1, INF)
        used = np.zeros(n + 1, dtype=bool)
        while True:
            used[j0] = True
            i0 = p[j0]
            cur = c[i0 - 1, :] - u[i0] - v[1:]
            free = ~used[1:]
            better = free & (cur < minv[1:])
            minv[1:][better] = cur[better]
            way[1:][better] = j0
            masked = np.where(free, minv[1:], INF)
            j1 = int(np.argmin(masked)) + 1
            delta = masked[j1 - 1]
            u[p[used]] += delta
            v[used] -= delta
            minv[1:][free] -= delta
            j0 = j1
            if p[j0] == 0:
                break
        while j0:
            j1 = way[j0]
            p[j0] = p[j1]
            j0 = j1
    col4row = np.zeros(n, dtype=np.int64)
    for j in range(1, n + 1):
        col4row[p[j] - 1] = j - 1
    return col4row


def _assign(sim_b):
    """col4row for maximizing sim_b (minimize -sim_b)."""
    try:
        from scipy.optimize import linear_sum_assignment

        r, c = linear_sum_assignment(-np.asarray(sim_b, dtype=np.float64))
        col = np.empty(sim_b.shape[0], dtype=np.int64)
        col[r] = c
        return col
    except ImportError:
        return _hungarian_min(-np.asarray(sim_b, dtype=np.float64))


def kernel(student_features, teacher_features):
    from concourse.bass_utils import run_bass_kernel_spmd

    s = np.asarray(student_features, dtype=np.float32)
    t = np.asarray(teacher_features, dtype=np.float32)

    sh = _l2norm(s)  # (B, N, D) f32
    th = _l2norm(t)

    # (B, 2, D, N) bf16, feature-major for the PE contraction
    x = np.stack(
        [
            np.ascontiguousarray(sh.transpose(0, 2, 1)),
            np.ascontiguousarray(th.transpose(0, 2, 1)),
        ],
        axis=1,
    ).astype(ml_dtypes.bfloat16)

    nc = _build_bass()
    in_maps = [{"x": x[c * BPC : (c + 1) * BPC]} for c in range(N_CORES)]
    res = run_bass_kernel_spmd(nc, in_maps, list(range(N_CORES)))
    sim = np.concatenate(
        [np.asarray(res.results[c]["sim"], dtype=np.float32) for c in range(N_CORES)],
        axis=0,
    )  # (B, N, N) f32 (from bf16)

    # Hungarian on device sims; exact f32 gather of the matched cosine values.
    total = 0.0
    for b in range(B):
        col = _assign(sim[b])
        # matched[i] = <sh[b,i], th[b,col[i]]> recomputed exactly
        total += np.einsum(
            "nd,nd->n", sh[b].astype(np.float64), th[b][col].astype(np.float64)
        ).sum()
    loss = -(total / (B * N))
    return np.float32(loss)


# revision 4
# speedup vs baseline: 1.2075x; 1.2075x over previous
"""CrossMatchingLoss Trainium2 kernel.

Problem: loss = -mean(matched cosine sims) where the matching is an exact
Hungarian assignment per batch element (detached / solved on CPU, exactly as
the reference does).

Split of work:
  host : L2-normalize (f32), transpose features to (D, N) layout, cast bf16,
         shard batches 4-per-core across 8 NeuronCores.
  device (per core): sim[b] = s_hat[b] @ t_hat[b]^T for its 4 batches as
         bf16 matmuls with f32 PSUM accumulation -> bf16 sim matrices.
  host : Hungarian assignment per batch on the device sim, then gather the
         matched cosine values (recomputed in f32/f64 for exactness) and
         average.

Shapes are hardcoded for B=32, N=256, D=1024, 8 cores (4 batches per core).
"""

import numpy as np
import ml_dtypes

B, N, D = 32, 256, 1024
N_CORES = 8
BPC = B // N_CORES          # batches per core
KC = D // 128               # contraction k-chunks of 128

_NC_CACHE = {}


def _build_bass():
    """Build (once) the per-core Bass program: 4x [256,1024]@[1024,256]^T."""
    if "nc" in _NC_CACHE:
        return _NC_CACHE["nc"]

    import concourse.bacc as bacc
    import concourse.mybir as mybir
    import concourse.tile as tile

    nc = bacc.Bacc("TRN2")
    # p-major packing: x[b, p, st, k, :] = feat_st[b, k*128 + p, :] so each
    # partition's DMA data is one contiguous 8KB run in HBM.
    x = nc.dram_tensor(
        "x", [BPC, 128, 2, KC, N], mybir.dt.bfloat16, kind="ExternalInput"
    )
    sim = nc.dram_tensor(
        "sim", [BPC, 128, 2, N], mybir.dt.bfloat16, kind="ExternalOutput"
    )

    with tile.TileContext(nc) as tc:
        with (
            tc.tile_pool(name="xin", bufs=3) as xin_pool,
            tc.tile_pool(name="so", bufs=2) as so_pool,
            tc.tile_pool(name="ps", bufs=4, space="PSUM") as ps_pool,
        ):
            for b in range(BPC):
                xt = xin_pool.tile([128, 2, KC, N], mybir.dt.bfloat16)
                nc.sync.dma_start(out=xt[:], in_=x[b])
                so = so_pool.tile([128, 2, N], mybir.dt.bfloat16)
                for j in range(2):
                    ps = ps_pool.tile([128, N], mybir.dt.float32)
                    for k in range(KC):
                        nc.tensor.matmul(
                            ps[:],
                            xt[:, 0, k, 128 * j : 128 * (j + 1)],  # lhsT [d,n]
                            xt[:, 1, k, :],                        # rhs  [d,m]
                            start=(k == 0),
                            stop=(k == KC - 1),
                        )
                    nc.vector.tensor_copy(so[:, j, :], ps[:])
                nc.scalar.dma_start(out=sim[b], in_=so[:])

    nc.compile()
    _NC_CACHE["nc"] = nc
    return nc


def _l2norm(x):
    n = np.sqrt(np.sum(np.square(x), axis=-1, keepdims=True, dtype=np.float32))
    return x / np.maximum(n, np.float32(1e-12))


def _pack_inputs(sh, th):
    """(B,N,D) f32 x2 -> (B, 128, 2, KC, N) bf16, p-major feature layout.

    x[b, p, st, k, :] = feat_st[b, :, k*128 + p] -- i.e. the (D, N)-transposed
    features with d split as (k, p) and p moved to the partition axis.
    """
    # (B, 2, N, D) -> (B, 2, N, KC, 128) -> (B, 128, 2, KC, N)
    f = np.stack([sh, th], axis=1).reshape(B, 2, N, KC, 128)
    return np.ascontiguousarray(f.transpose(0, 4, 1, 3, 2)).astype(
        ml_dtypes.bfloat16
    )


def _hungarian_min(cost):
    """Exact square linear_sum_assignment (minimize); numpy fallback."""
    n = cost.shape[0]
    c = np.asarray(cost, dtype=np.float64)
    INF = np.inf
    u = np.zeros(n + 1)
    v = np.zeros(n + 1)
    p = np.zeros(n + 1, dtype=np.int64)
    way = np.zeros(n + 1, dtype=np.int64)
    for i in range(1, n + 1):
        p[0] = i
        j0 = 0
        minv = np.full(n + 1, INF)
        used = np.zeros(n + 1, dtype=bool)
        while True:
            used[j0] = True
            i0 = p[j0]
            cur = c[i0 - 1, :] - u[i0] - v[1:]
            free = ~used[1:]
            better = free & (cur < minv[1:])
            minv[1:][better] = cur[better]
            way[1:][better] = j0
            masked = np.where(free, minv[1:], INF)
            j1 = int(np.argmin(masked)) + 1
            delta = masked[j1 - 1]
            u[p[used]] += delta
            v[used] -= delta
            minv[1:][free] -= delta
            j0 = j1
            if p[j0] == 0:
                break
        while j0:
            j1 = way[j0]
            p[j0] = p[j1]
            j0 = j1
    col4row = np.zeros(n, dtype=np.int64)
    for j in range(1, n + 1):
        col4row[p[j] - 1] = j - 1
    return col4row


def _assign(sim_b):
    """col4row for maximizing sim_b (minimize -sim_b)."""
    try:
        from scipy.optimize import linear_sum_assignment

        r, c = linear_sum_assignment(-np.asarray(sim_b, dtype=np.float64))
        col = np.empty(sim_b.shape[0], dtype=np.int64)
        col[r] = c
        return col
    except ImportError:
        return _hungarian_min(-np.asarray(sim_b, dtype=np.float64))


def kernel(student_features, teacher_features):
    from concourse.bass_utils import run_bass_kernel_spmd

    s = np.asarray(student_features, dtype=np.float32)
    t = np.asarray(teacher_features, dtype=np.float32)

    sh = _l2norm(s)  # (B, N, D) f32
    th = _l2norm(t)

    x = _pack_inputs(sh, th)
    nc = _build_bass()
    in_maps = [{"x": x[c * BPC : (c + 1) * BPC]} for c in range(N_CORES)]
    res = run_bass_kernel_spmd(nc, in_maps, list(range(N_CORES)))
    # device sim layout: (BPC, p, j, m) with n = j*128 + p
    sim = np.concatenate(
        [np.asarray(res.results[c]["sim"]) for c in range(N_CORES)], axis=0
    )  # (B, 128, 2, N) bf16
    sim = sim.transpose(0, 2, 1, 3).reshape(B, N, N).astype(np.float32)

    # Hungarian on device sims; exact f32 gather of the matched cosine values.
    total = 0.0
    for b in range(B):
        col = _assign(sim[b])
        # matched[i] = <sh[b,i], th[b,col[i]]> recomputed exactly
        total += np.einsum(
            "nd,nd->n", sh[b].astype(np.float64), th[b][col].astype(np.float64)
        ).sum()
    loss = -(total / (B * N))
    return np.float32(loss)


# revision 7
# speedup vs baseline: 1.2319x; 1.0202x over previous
"""CrossMatchingLoss Trainium2 kernel.

Problem: loss = -mean(matched cosine sims) where the matching is an exact
Hungarian assignment per batch element (detached / solved on CPU, exactly as
the reference does).

Split of work:
  host : L2-normalize (f32), transpose features to (D, N) layout, cast bf16,
         shard batches 4-per-core across 8 NeuronCores.
  device (per core): sim[b] = s_hat[b] @ t_hat[b]^T for its 4 batches as
         bf16 matmuls with f32 PSUM accumulation -> bf16 sim matrices.
  host : Hungarian assignment per batch on the device sim, then gather the
         matched cosine values (recomputed in f32/f64 for exactness) and
         average.

Shapes are hardcoded for B=32, N=256, D=1024, 8 cores (4 batches per core).
"""

import numpy as np
import ml_dtypes

B, N, D = 32, 256, 1024
N_CORES = 8
BPC = B // N_CORES          # batches per core
KC = D // 128               # contraction k-chunks of 128

_NC_CACHE = {}


def _build_bass():
    """Build (once) the per-core Bass program: 4x [256,1024]@[1024,256]^T."""
    if "nc" in _NC_CACHE:
        return _NC_CACHE["nc"]

    import concourse.bacc as bacc
    import concourse.mybir as mybir
    import concourse.tile as tile

    nc = bacc.Bacc("TRN2")
    # p-major packing split in two k-halves: x[b, h, p, st, kk, :] =
    # feat_st[b, (h*4 + kk)*128 + p, :]. Each partition's data per DMA is one
    # contiguous 4KB run; the two halves load on different HWDGE rings.
    KH = KC // 2  # 4 k-chunks per half
    x = nc.dram_tensor(
        "x", [BPC, 2, 128, 2, KH, N], mybir.dt.bfloat16, kind="ExternalInput"
    )
    sim = nc.dram_tensor(
        "sim", [BPC, 128, 2, N], mybir.dt.bfloat16, kind="ExternalOutput"
    )

    with tile.TileContext(nc) as tc:
        with (
            tc.tile_pool(name="xin", bufs=4) as xin_pool,
            tc.tile_pool(name="so", bufs=2) as so_pool,
            tc.tile_pool(name="ps", bufs=4, space="PSUM") as ps_pool,
        ):
            for b in range(BPC):
                xh0 = xin_pool.tile([128, 2, KH, N], mybir.dt.bfloat16, tag="h0")
                xh1 = xin_pool.tile([128, 2, KH, N], mybir.dt.bfloat16, tag="h1")
                nc.sync.dma_start(out=xh0[:], in_=x[b, 0])
                nc.scalar.dma_start(out=xh1[:], in_=x[b, 1])
                so = so_pool.tile([128, 2, N], mybir.dt.bfloat16)
                ps = [
                    ps_pool.tile(
                        [128, N], mybir.dt.float32, tag=f"ps{j}", name=f"ps{j}"
                    )
                    for j in range(2)
                ]
                for k in range(KC):
                    h, kk = divmod(k, KH)
                    xt = xh1 if h else xh0
                    for j in range(2):
                        nc.tensor.matmul(
                            ps[j][:],
                            xt[:, 0, kk, 128 * j : 128 * (j + 1)],  # lhsT [d,n]
                            xt[:, 1, kk, :],                        # rhs  [d,m]
                            start=(k == 0),
                            stop=(k == KC - 1),
                        )
                for j in range(2):
                    nc.vector.tensor_copy(so[:, j, :], ps[j][:])
                nc.gpsimd.dma_start(out=sim[b], in_=so[:])

    nc.compile()
    _NC_CACHE["nc"] = nc
    return nc


def _l2norm(x):
    n = np.sqrt(np.sum(np.square(x), axis=-1, keepdims=True, dtype=np.float32))
    return x / np.maximum(n, np.float32(1e-12))


def _pack_inputs(sh, th):
    """(B,N,D) f32 x2 -> (B, 2, 128, 2, KC//2, N) bf16 p-major feature layout.

    x[b, h, p, st, kk, :] = feat_st[b, :, (h*KC//2 + kk)*128 + p] -- the
    (D, N)-transposed features with d split as (h, kk, p), p on partitions.
    """
    KH = KC // 2
    # (B, 2st, N, D) -> (B, 2st, N, 2h, KH, 128p) -> (B, 2h, 128p, 2st, KH, N)
    f = np.stack([sh, th], axis=1).reshape(B, 2, N, 2, KH, 128)
    return np.ascontiguousarray(f.transpose(0, 3, 5, 1, 4, 2)).astype(
        ml_dtypes.bfloat16
    )


def _hungarian_min(cost):
    """Exact square linear_sum_assignment (minimize); numpy fallback."""
    n = cost.shape[0]
    c = np.asarray(cost, dtype=np.float64)
    INF = np.inf
    u = np.zeros(n + 1)
    v = np.zeros(n + 1)
    p = np.zeros(n + 1, dtype=np.int64)
    way = np.zeros(n + 1, dtype=np.int64)
    for i in range(1, n + 1):
        p[0] = i
        j0 = 0
        minv = np.full(n + 1, INF)
        used = np.zeros(n + 1, dtype=bool)
        while True:
            used[j0] = True
            i0 = p[j0]
            cur = c[i0 - 1, :] - u[i0] - v[1:]
            free = ~used[1:]
            better = free & (cur < minv[1:])
            minv[1:][better] = cur[better]
            way[1:][better] = j0
            masked = np.where(free, minv[1:], INF)
            j1 = int(np.argmin(masked)) + 1
            delta = masked[j1 - 1]
            u[p[used]] += delta
            v[used] -= delta
            minv[1:][free] -= delta
            j0 = j1
            if p[j0] == 0:
                break
        while j0:
            j1 = way[j0]
            p[j0] = p[j1]
            j0 = j1
    col4row = np.zeros(n, dtype=np.int64)
    for j in range(1, n + 1):
        col4row[p[j] - 1] = j - 1
    return col4row


def _assign(sim_b):
    """col4row for maximizing sim_b (minimize -sim_b)."""
    try:
        from scipy.optimize import linear_sum_assignment

        r, c = linear_sum_assignment(-np.asarray(sim_b, dtype=np.float64))
        col = np.empty(sim_b.shape[0], dtype=np.int64)
        col[r] = c
        return col
    except ImportError:
        return _hungarian_min(-np.asarray(sim_b, dtype=np.float64))


def kernel(student_features, teacher_features):
    from concourse.bass_utils import run_bass_kernel_spmd

    s = np.asarray(student_features, dtype=np.float32)
    t = np.asarray(teacher_features, dtype=np.float32)

    sh = _l2norm(s)  # (B, N, D) f32
    th = _l2norm(t)

    x = _pack_inputs(sh, th)
    nc = _build_bass()
    in_maps = [{"x": x[c * BPC : (c + 1) * BPC]} for c in range(N_CORES)]
    res = run_bass_kernel_spmd(nc, in_maps, list(range(N_CORES)))
    # device sim layout: (BPC, p, j, m) with n = j*128 + p
    sim = np.concatenate(
        [np.asarray(res.results[c]["sim"]) for c in range(N_CORES)], axis=0
    )  # (B, 128, 2, N) bf16
    sim = sim.transpose(0, 2, 1, 3).reshape(B, N, N).astype(np.float32)

    # Hungarian on device sims; exact f32 gather of the matched cosine values.
    total = 0.0
    for b in range(B):
        col = _assign(sim[b])
        # matched[i] = <sh[b,i], th[b,col[i]]> recomputed exactly
        total += np.einsum(
            "nd,nd->n", sh[b].astype(np.float64), th[b][col].astype(np.float64)
        ).sum()
    loss = -(total / (B * N))
    return np.float32(loss)
